# revision 9
# baseline (speedup 1.0000x reference)
"""Trainium2 Bass kernel for nn_MILPFAttnTrexModel (segment_reduce).

Contract: kernel(**inputs) takes the FULL unsharded inputs (numpy arrays, keys
as in reference.setup_inputs()) and returns the FULL [G, NC] float32 output.

Strategy (8 NeuronCores, SPMD — one program, per-core data):
  - Host buckets rows by group; 8 groups per core, each group's tile rows
    padded to a uniform TB-column block (TB multiple of 384). Inputs ship
    pre-transposed (feature-major) so K lands on SBUF partitions.
  - L1 MLP in fp8 e4m3 DoubleRow (x, Wl0 quantized with pow2 scales on host;
    dequant folded into bias / downstream weights so PSUM evictions stay
    2-op). FD=256 chunks; chains run in three parallel PSUM banks so each
    stationary is streamed by consecutive matmuls, the repeats marked
    ldweights=False to skip redundant PE weight loads.
  - L2 bf16, same parallel-bank chaining, writes xt2 directly as fp8.
  - v and transposed scores per 128-row chunk from ONE fp8 DoubleRow
    stationary (the xt2 chunk; the scores matmul reuses it via
    ldweights=False) streaming Wv then Wk@latent.T into a single PSUM tile.
  - Softmax without max-subtraction (scores are O(1)); per-chunk Exp with a
    per-partition bias that kills pad rows; the attention denominator comes
    free from a ones column prepended to v in the weighted-sum matmul;
    normalization and fp8 dequant fold into the final per-group scale.
  - Whole-image branch: bf16 MLP on the 8 whole rows per core (WB == 1, so
    segment_max is the row itself). Its weights are DMA'd up front to
    overlap the tile branch.
  - Host: bv add, assembly, final fused @ Wout + bout.
"""

import math
import os
import numpy as np
import ml_dtypes

import concourse.bacc as bacc
import concourse.tile as tile
from concourse import mybir
from concourse.bass_utils import run_bass_kernel_spmd

# Set by the most recent kernel() call when KERNEL_TRACE=1 (dev-only).
last_exec_time_ns = None
last_mean_exec_time_ns = None


def _install_ntff_shim():
    """Register the axon NTFF profile hook if the image's antenv lacks it."""
    import sys, types
    try:
        import antenv.axon_hooks  # noqa: F401
        return
    except ImportError:
        pass
    m = types.ModuleType("antenv.axon_hooks")
    m._hook = None
    m.set_axon_ntff_profile_hook = lambda h: setattr(m, "_hook", h)
    m.get_axon_ntff_profile_hook = lambda: m._hook
    sys.modules["antenv.axon_hooks"] = m
    import antenv
    antenv.axon_hooks = m
    from trn_agent_boot.trn_boot import _ntff_profile_via_ctypes
    m.set_axon_ntff_profile_hook(
        _ntff_profile_via_ctypes("/opt/axon/libaxon_pjrt.so"))

F32 = mybir.dt.float32
BF16 = mybir.dt.bfloat16
FP8 = mybir.dt.float8e4
AX = mybir.AxisListType
ALU = mybir.AluOpType
ACTF = mybir.ActivationFunctionType
DR = mybir.MatmulPerfMode.DoubleRow

NP_FP8 = ml_dtypes.float8_e4m3
NP_BF16 = ml_dtypes.bfloat16

N_CORES = 8
G = 64
GPC = G // N_CORES          # groups per core
IN = 1024
GL = 512
LC = 256
L = 8
NCLS = 2

# power-of-2 quantization scales
SX = 16.0          # x fp8 payload = SX * x
SW0 = 2048.0       # Wl0 fp8 payload = SW0 * Wl0   -> L1 psum = 32768 * (x@Wl0)
C1 = SX * SW0      # 32768
SH2 = 16.0         # xt2 fp8 payload = SH2 * xt2 (via Wl1 scaled SH2/C1)
SWV = 1024.0       # Wv fp8 payload = SWV * Wv     -> v psum = SH2*SWV * v
SKL = 512.0        # wkl fp8 payload = SKL * wkl   -> sct psum = SH2*SKL * s
PADB = -100.0      # Exp bias on pad rows: exp(s - 100) flushes to 0 in bf16

SKIP_LDW = os.environ.get("KERNEL_NO_SKIP_LDW") != "1"

_prog_cache = {}


def _ceil_to(x, m):
    return ((x + m - 1) // m) * m


def _build_program(TB, skip_ldw):
    """Build the SPMD Tile program for tile-block size TB (multiple of 384)."""
    T = GPC * TB
    RC = TB // 128           # 128-row chunks per group
    # L1 fp8-DR chunks (FD<=256) grouped into PSUM tiles of 512 output cols
    # tile k covers h1 cols [k*512, k*512+512) as two sequential 256-chains;
    # a ragged tail tile covers the rest.
    l1_full = TB // 512
    l1_tail = TB - l1_full * 512          # < 512, multiple of 128
    # chunk plan per phase: (tile_idx, col_off_in_tile, col_off_in_out, width)
    l1_ph0, l1_ph1 = [], []
    for k in range(l1_full):
        l1_ph0.append((k, 0, k * 512, 256))
        l1_ph1.append((k, 256, k * 512 + 256, 256))
    if l1_tail:
        t0 = min(l1_tail, 256)
        l1_ph0.append((l1_full, 0, l1_full * 512, t0))
        if l1_tail > t0:
            l1_ph1.append((l1_full, t0, l1_full * 512 + t0, l1_tail - t0))
    l1_ntiles = l1_full + (1 if l1_tail else 0)
    # L2 bf16 chunks of FD=384
    NC2 = TB // 384

    nc = bacc.Bacc("TRN2", target_bir_lowering=False, debug=False,
                   num_devices=N_CORES)

    xtq = nc.dram_tensor("xtq", [IN, T], FP8, kind="ExternalInput")
    padb = nc.dram_tensor("padb", [128, GPC * RC], F32, kind="ExternalInput")
    xw = nc.dram_tensor("xw", [IN, GPC], BF16, kind="ExternalInput")
    wl0q = nc.dram_tensor("wl0q", [IN, GL], FP8, kind="ExternalInput")
    wl1e = nc.dram_tensor("wl1e", [GL, LC], BF16, kind="ExternalInput")
    wvq = nc.dram_tensor("wvq", [LC, LC], FP8, kind="ExternalInput")
    wklq = nc.dram_tensor("wklq", [LC, L], FP8, kind="ExternalInput")
    wg0 = nc.dram_tensor("wg0", [IN, 2 * GL], BF16, kind="ExternalInput")
    wg1 = nc.dram_tensor("wg1", [2 * GL, GL], BF16, kind="ExternalInput")
    bl0s = nc.dram_tensor("bl0s", [128, GL // 128], F32, kind="ExternalInput")
    bl1s = nc.dram_tensor("bl1s", [128, LC // 128], F32, kind="ExternalInput")
    bg0t = nc.dram_tensor("bg0t", [128, 2 * GL // 128], F32, kind="ExternalInput")
    bg1t = nc.dram_tensor("bg1t", [128, GL // 128], F32, kind="ExternalInput")
    out_og = nc.dram_tensor("out_og", [L, GPC, LC], F32, kind="ExternalOutput")
    out_w = nc.dram_tensor("out_w", [128, GL // 128, GPC], F32,
                           kind="ExternalOutput")

    tick = [0]

    def evac(out_ap, in_ap, bias_ap=None):
        """PSUM -> SBUF eviction; optional fused bias-add + relu.
        Alternates DVE / ACT to balance engine load."""
        use_dve = (tick[0] % 2 == 0)
        tick[0] += 1
        if bias_ap is None:
            if use_dve:
                nc.vector.tensor_copy(out_ap, in_ap)
            else:
                nc.scalar.copy(out_ap, in_ap)
        else:
            if use_dve:
                nc.vector.tensor_scalar(out_ap, in_ap, bias_ap, 0.0,
                                        op0=ALU.add, op1=ALU.max)
            else:
                nc.scalar.activation(out_ap, in_ap, ACTF.Relu, bias=bias_ap)

    def mm(out_ap, lhsT, rhs, start, stop, perf_mode=None, reuse=False):
        bi = nc.tensor.matmul(out_ap, lhsT, rhs, start=start, stop=stop,
                              perf_mode=perf_mode)
        if reuse and skip_ldw:
            bi.ins.ldweights = False
        return bi

    with tile.TileContext(nc) as tc:
        with (
            tc.tile_pool(name="weights", bufs=1) as wpool,
            tc.tile_pool(name="wg", bufs=1) as wgpool,
        ):
            wl0_sb = wpool.tile([128, IN // 128, GL], FP8)
            nc.scalar.dma_start(out=wl0_sb,
                                in_=wl0q.ap().rearrange("(kt p) m -> p kt m",
                                                        p=128))
            wl1_sb = wpool.tile([128, GL // 128, LC], BF16)
            nc.scalar.dma_start(out=wl1_sb,
                                in_=wl1e.ap().rearrange("(kt p) m -> p kt m",
                                                        p=128))
            wv_sb = wpool.tile([128, LC // 128, LC], FP8)
            nc.scalar.dma_start(out=wv_sb,
                                in_=wvq.ap().rearrange("(kt p) m -> p kt m",
                                                       p=128))
            wkl_sb = wpool.tile([128, LC // 128, L], FP8)
            nc.scalar.dma_start(out=wkl_sb,
                                in_=wklq.ap().rearrange("(kt p) m -> p kt m",
                                                        p=128))
            bl0_sb = wpool.tile([128, GL // 128], F32)
            nc.scalar.dma_start(out=bl0_sb, in_=bl0s.ap())
            bl1_sb = wpool.tile([128, LC // 128], F32)
            nc.scalar.dma_start(out=bl1_sb, in_=bl1s.ap())
            padb_sb = wpool.tile([128, GPC * RC], F32)
            nc.scalar.dma_start(out=padb_sb, in_=padb.ap())
            # whole-branch weights up front so their DMA overlaps tile compute
            wg0_sb = wgpool.tile([128, IN // 128, 2 * GL], BF16)
            nc.gpsimd.dma_start(out=wg0_sb,
                                in_=wg0.ap().rearrange("(kt p) m -> p kt m",
                                                       p=128))
            wg1_sb = wgpool.tile([128, 2 * GL // 128, GL], BF16)
            nc.gpsimd.dma_start(out=wg1_sb,
                                in_=wg1.ap().rearrange("(kt p) m -> p kt m",
                                                       p=128))
            bg0_sb = wgpool.tile([128, 2 * GL // 128], F32)
            nc.gpsimd.dma_start(out=bg0_sb, in_=bg0t.ap())
            bg1_sb = wgpool.tile([128, GL // 128], F32)
            nc.gpsimd.dma_start(out=bg1_sb, in_=bg1t.ap())
            xw_sb = wgpool.tile([128, IN // 128, GPC], BF16)
            nc.gpsimd.dma_start(out=xw_sb,
                                in_=xw.ap().rearrange("(kt p) t -> p kt t",
                                                      p=128))

            # ---------------- tile-instance branch, per group ----------------
            with (
                tc.tile_pool(name="xt", bufs=2) as xtpool,
                tc.tile_pool(name="h1", bufs=2) as h1pool,
                tc.tile_pool(name="xt2", bufs=2) as xt2pool,
                tc.tile_pool(name="vall", bufs=2) as vpool,
                tc.tile_pool(name="ext", bufs=2) as extpool,
                tc.tile_pool(name="small", bufs=2) as smpool,
                tc.tile_pool(name="ogall", bufs=1) as ogpool,
                tc.tile_pool(name="pA", bufs=3, space="PSUM") as pA,
                tc.tile_pool(name="pT", bufs=1, space="PSUM") as pT,
                tc.tile_pool(name="pV", bufs=2, space="PSUM") as pV,
                tc.tile_pool(name="pO", bufs=2, space="PSUM") as pO,
            ):
                og_sb = ogpool.tile([L, GPC, LC], F32)
                xtq_r = xtq.ap().rearrange("(kt p) t -> p kt t", p=128)
                KP = IN // 256

                for j in range(GPC):
                    c0 = j * TB
                    xt_sb = xtpool.tile([128, IN // 128, TB], FP8)
                    nc.sync.dma_start(out=xt_sb, in_=xtq_r[:, :, c0:c0 + TB])

                    # L1: h1' = relu(32768*(x@Wl0) + 32768*bl0)  [512, TB] bf16
                    # chains run in parallel banks; same stationary streamed
                    # back-to-back with ldweights skipped on the repeats.
                    h1_sb = h1pool.tile([128, GL // 128, TB], BF16)
                    for mc in range(GL // 128):
                        tiles = [pA.tile([128, 512], F32, tag="pa", name="pa")
                                 for _ in range(min(l1_ntiles, l1_full))]
                        if l1_tail:
                            tiles.append(pT.tile([128, 512], F32, tag="pt",
                                                 name="pt"))
                        for phase in (l1_ph0, l1_ph1):
                            for kp in range(KP):
                                st, sp = kp == 0, kp == KP - 1
                                for i, (tk, toff, ooff, w) in enumerate(phase):
                                    mm(tiles[tk][:, toff:toff + w],
                                       wl0_sb[:, 2 * kp:2 * kp + 2,
                                              mc * 128:(mc + 1) * 128],
                                       xt_sb[:, 2 * kp:2 * kp + 2,
                                             ooff:ooff + w],
                                       st, sp, DR, reuse=(i > 0))
                        for k in range(l1_full):
                            evac(h1_sb[:, mc, k * 512:(k + 1) * 512], tiles[k],
                                 bl0_sb[:, mc:mc + 1])
                        if l1_tail:
                            evac(h1_sb[:, mc, l1_full * 512:TB],
                                 tiles[-1][:, 0:l1_tail], bl0_sb[:, mc:mc + 1])

                    # L2: xt2' = relu(16*(h1@Wl1) + 16*bl1) [256, TB] fp8
                    xt2_sb = xt2pool.tile([128, LC // 128, TB], FP8)
                    for mc in range(LC // 128):
                        tiles = [pA.tile([128, 512], F32, tag="pa", name="pa")
                                 for _ in range(min(NC2, 2))]
                        for _ in range(NC2 - 2):
                            tiles.append(pT.tile([128, 512], F32, tag="pt",
                                                 name="pt"))
                        for kt in range(GL // 128):
                            st, sp = kt == 0, kt == GL // 128 - 1
                            for t in range(NC2):
                                mm(tiles[t][:, 0:384],
                                   wl1_sb[:, kt, mc * 128:(mc + 1) * 128],
                                   h1_sb[:, kt, t * 384:(t + 1) * 384],
                                   st, sp, reuse=(t > 0))
                        for t in range(NC2):
                            evac(xt2_sb[:, mc, t * 384:(t + 1) * 384],
                                 tiles[t][:, 0:384], bl1_sb[:, mc:mc + 1])

                    # v' and transposed scores, per 128-row chunk:
                    # pv[:, 0:256]   = 16384 * v_rows
                    # pv[:, 256:264] = 8192 * scores_rows
                    vall_sb = vpool.tile([128, RC, 1 + LC + L], BF16)
                    nc.vector.memset(vall_sb[:, :, 0:1], 1.0)
                    ext_sb = extpool.tile([128, RC, L], BF16)
                    for rc in range(RC):
                        r0 = rc * 128
                        pv = pV.tile([128, LC + L], F32, tag="pv")
                        mm(pv[:, 0:LC], xt2_sb[:, 0:2, r0:r0 + 128],
                           wv_sb[:, 0:2, :], True, True, DR)
                        mm(pv[:, LC:LC + L], xt2_sb[:, 0:2, r0:r0 + 128],
                           wkl_sb[:, 0:2, :], True, True, DR, reuse=True)
                        evac(vall_sb[:, rc, 1:1 + LC + L], pv)
                        # ex = exp(scores + padkill) (bf16); pads -> 0
                        nc.scalar.activation(
                            ext_sb[:, rc, :], vall_sb[:, rc, 1 + LC:1 + LC + L],
                            ACTF.Exp, scale=1.0 / (SH2 * SKL),
                            bias=padb_sb[:, j * RC + rc:j * RC + rc + 1])

                    # out_cat[l, 0] = denom, out_cat[l, 1:] = 16384*sum ex*v
                    po = pO.tile([L, 1 + LC], F32, tag="po")
                    for rc in range(RC):
                        nc.tensor.matmul(po, ext_sb[:, rc, :],
                                         vall_sb[:, rc, 0:1 + LC],
                                         start=(rc == 0), stop=(rc == RC - 1))
                    rden = smpool.tile([L, 1], F32, tag="rden")
                    nc.vector.reciprocal(rden, po[:, 0:1])
                    nc.vector.tensor_scalar(og_sb[:, j, :], po[:, 1:1 + LC],
                                            rden, 1.0 / (SH2 * SWV),
                                            op0=ALU.mult, op1=ALU.mult)

                nc.sync.dma_start(out=out_og.ap(), in_=og_sb)

            # ---------------- whole-instance branch (WB == 1) ----------------
            with (
                tc.tile_pool(name="wtile", bufs=1) as wtpool,
                tc.tile_pool(name="pw", bufs=2, space="PSUM") as pw,
            ):
                h1w_sb = wtpool.tile([128, 2 * GL // 128, GPC], BF16)
                for mc in range(2 * GL // 128):
                    ps = pw.tile([128, GPC], F32, tag="pw")
                    for kt in range(IN // 128):
                        nc.tensor.matmul(
                            ps, wg0_sb[:, kt, mc * 128:(mc + 1) * 128],
                            xw_sb[:, kt, :],
                            start=(kt == 0), stop=(kt == IN // 128 - 1))
                    evac(h1w_sb[:, mc, :], ps, bg0_sb[:, mc:mc + 1])

                wag_sb = wtpool.tile([128, GL // 128, GPC], F32)
                for mc in range(GL // 128):
                    ps = pw.tile([128, GPC], F32, tag="pw")
                    for kt in range(2 * GL // 128):
                        nc.tensor.matmul(
                            ps, wg1_sb[:, kt, mc * 128:(mc + 1) * 128],
                            h1w_sb[:, kt, :],
                            start=(kt == 0), stop=(kt == 2 * GL // 128 - 1))
                    evac(wag_sb[:, mc, :], ps, bg1_sb[:, mc:mc + 1])
                nc.sync.dma_start(out=out_w.ap(), in_=wag_sb)

    nc.compile()
    return nc


def _get_program(key):
    if key not in _prog_cache:
        _prog_cache[key] = _build_program(*key)
    return _prog_cache[key]


def kernel(**inputs):
    x = np.ascontiguousarray(np.asarray(inputs["x"], dtype=np.float32))
    group = np.asarray(inputs["group"]).astype(np.int64)
    itype = np.asarray(inputs["instance_type"]).astype(np.int64)
    Wl0 = np.asarray(inputs["Wl0"], np.float32)
    bl0 = np.asarray(inputs["bl0"], np.float32)
    Wl1 = np.asarray(inputs["Wl1"], np.float32)
    bl1 = np.asarray(inputs["bl1"], np.float32)
    Wg0 = np.asarray(inputs["Wg0"], np.float32)
    bg0 = np.asarray(inputs["bg0"], np.float32)
    Wg1 = np.asarray(inputs["Wg1"], np.float32)
    bg1 = np.asarray(inputs["bg1"], np.float32)
    Wk = np.asarray(inputs["Wk"], np.float32)
    Wv = np.asarray(inputs["Wv"], np.float32)
    bv = np.asarray(inputs["bv"], np.float32)
    latent = np.asarray(inputs["latent"], np.float32)
    Wout = np.asarray(inputs["Wout"], np.float32)
    bout = np.asarray(inputs["bout"], np.float32)

    # ---- host bucketing ----
    is_tile = itype == 1
    is_whole = itype == 0
    tile_idx = [np.where(is_tile & (group == g))[0] for g in range(G)]
    whole_idx = [np.where(is_whole & (group == g))[0] for g in range(G)]
    ng = np.array([len(ix) for ix in tile_idx])
    wg = np.array([len(ix) for ix in whole_idx])
    assert (wg == 1).all(), "kernel assumes exactly one whole instance per group"
    TB = max(384, _ceil_to(int(ng.max()), 384))
    T = GPC * TB
    RC = TB // 128

    # ---- per-core staged arrays ----
    scale = 1.0 / math.sqrt(LC)
    wkl = (Wk @ latent.T) * scale                        # [LC, L]
    shared = dict(
        wl0q=np.ascontiguousarray(Wl0 * SW0).astype(NP_FP8),
        wl1e=np.ascontiguousarray(Wl1 * (SH2 / C1)).astype(NP_BF16),
        wvq=np.ascontiguousarray(Wv * SWV).astype(NP_FP8),
        wklq=np.ascontiguousarray(wkl * SKL).astype(NP_FP8),
        wg0=np.ascontiguousarray(Wg0).astype(NP_BF16),
        wg1=np.ascontiguousarray(Wg1).astype(NP_BF16),
        bl0s=np.ascontiguousarray((bl0 * C1).reshape(-1, 128).T),
        bl1s=np.ascontiguousarray((bl1 * SH2).reshape(-1, 128).T),
        bg0t=np.ascontiguousarray(bg0.reshape(-1, 128).T),
        bg1t=np.ascontiguousarray(bg1.reshape(-1, 128).T),
    )
    in_maps = []
    for c in range(N_CORES):
        xtq = np.zeros((IN, T), np.float32)
        xwf = np.zeros((IN, GPC), np.float32)
        pb = np.full((128, GPC * RC), PADB, np.float32)
        for j in range(GPC):
            g = c * GPC + j
            ti, wi = tile_idx[g], whole_idx[g]
            xtq[:, j * TB:j * TB + len(ti)] = x[ti].T * SX
            xwf[:, j:j + 1] = x[wi].T
            n = len(ti)
            full_rc, rem = divmod(n, 128)
            pb[:, j * RC:j * RC + full_rc] = 0.0
            if rem:
                pb[:rem, j * RC + full_rc] = 0.0
        in_maps.append(dict(xtq=xtq.astype(NP_FP8), padb=pb,
                            xw=xwf.astype(NP_BF16), **shared))

    nc = _get_program((TB, SKIP_LDW))
    trace = os.environ.get("KERNEL_TRACE") == "1"
    if trace:
        _install_ntff_shim()
    res = run_bass_kernel_spmd(nc, in_maps, core_ids=list(range(N_CORES)),
                               trace=trace)
    global last_exec_time_ns, last_mean_exec_time_ns
    last_exec_time_ns = res.exec_time_ns
    last_mean_exec_time_ns = res.mean_exec_time_ns

    # ---- host assembly ----
    whole_agg = np.empty((G, GL), np.float32)
    out_group = np.empty((G, L, LC), np.float32)
    for c in range(N_CORES):
        ow = np.asarray(res.results[c]["out_w"], np.float32)
        og = np.asarray(res.results[c]["out_og"], np.float32)
        wa = ow.transpose(1, 0, 2).reshape(GL, GPC)
        for j in range(GPC):
            g = c * GPC + j
            whole_agg[g] = wa[:, j]
            out_group[g] = og[:, j, :] + bv[None, :]
    fused = np.concatenate([whole_agg, out_group.reshape(G, L * LC)], axis=1)
    return (fused @ Wout + bout).astype(np.float32)


# revision 10
# speedup vs baseline: 1.0184x; 1.0184x over previous
"""Trainium2 Bass kernel for nn_MILPFAttnTrexModel (segment_reduce).

Contract: kernel(**inputs) takes the FULL unsharded inputs (numpy arrays, keys
as in reference.setup_inputs()) and returns the FULL [G, NC] float32 output.

Strategy (8 NeuronCores, SPMD — one program, per-core data):
  - Host buckets rows by group; 8 groups per core, each group's tile rows
    padded to a uniform TB-column block (TB multiple of 384). All device
    inputs are staged partition-major with each partition's data contiguous,
    so every DMA moves KB-scale runs.
  - L1 MLP in fp8 e4m3 DoubleRow (x, Wl0 quantized with pow2 scales on host;
    dequant folded into bias / downstream weights so PSUM evictions stay
    2-op). FD=256 chunks; chains run in three parallel PSUM banks.
  - L2 bf16, same parallel-bank chaining, writes xt2 directly as fp8.
  - v and transposed scores per 128-row chunk from ONE fp8 DoubleRow
    stationary (the xt2 chunk) streaming Wv then Wk@latent.T into a single
    PSUM tile.
  - Softmax without max-subtraction (scores are O(1)); per-chunk Exp with a
    per-partition bias that kills pad rows; the attention denominator comes
    free from a ones column prepended to v in the weighted-sum matmul;
    normalization and fp8 dequant fold into the final per-group scale.
  - Whole-image branch (bf16 MLP on the 8 whole rows per core; WB == 1 so
    segment_max is the row itself) runs FIRST, filling the window while the
    big tile-branch DMAs land; its outputs leave early, trimming the tail.
  - Host: bv add, assembly, final fused @ Wout + bout.
"""

import math
import os
import numpy as np
import ml_dtypes

import concourse.bacc as bacc
import concourse.tile as tile
from concourse import mybir
from concourse.bass_utils import run_bass_kernel_spmd

# Set by the most recent kernel() call when KERNEL_TRACE=1 (dev-only).
last_exec_time_ns = None
last_mean_exec_time_ns = None


def _install_ntff_shim():
    """Register the axon NTFF profile hook if the image's antenv lacks it."""
    import sys, types
    try:
        import antenv.axon_hooks  # noqa: F401
        return
    except ImportError:
        pass
    m = types.ModuleType("antenv.axon_hooks")
    m._hook = None
    m.set_axon_ntff_profile_hook = lambda h: setattr(m, "_hook", h)
    m.get_axon_ntff_profile_hook = lambda: m._hook
    sys.modules["antenv.axon_hooks"] = m
    import antenv
    antenv.axon_hooks = m
    from trn_agent_boot.trn_boot import _ntff_profile_via_ctypes
    m.set_axon_ntff_profile_hook(
        _ntff_profile_via_ctypes("/opt/axon/libaxon_pjrt.so"))

F32 = mybir.dt.float32
BF16 = mybir.dt.bfloat16
FP8 = mybir.dt.float8e4
AX = mybir.AxisListType
ALU = mybir.AluOpType
ACTF = mybir.ActivationFunctionType
DR = mybir.MatmulPerfMode.DoubleRow

NP_FP8 = ml_dtypes.float8_e4m3
NP_BF16 = ml_dtypes.bfloat16

N_CORES = 8
G = 64
GPC = G // N_CORES          # groups per core
IN = 1024
GL = 512
LC = 256
L = 8
NCLS = 2

# power-of-2 quantization scales
SX = 16.0          # x fp8 payload = SX * x
SW0 = 2048.0       # Wl0 fp8 payload = SW0 * Wl0   -> L1 psum = 32768 * (x@Wl0)
C1 = SX * SW0      # 32768
SH2 = 16.0         # xt2 fp8 payload = SH2 * xt2 (via Wl1 scaled SH2/C1)
SWV = 1024.0       # Wv fp8 payload = SWV * Wv     -> v psum = SH2*SWV * v
SKL = 512.0        # wkl fp8 payload = SKL * wkl   -> sct psum = SH2*SKL * s
PADB = -100.0      # Exp bias on pad rows: exp(s - 100) flushes to 0 in bf16

_prog_cache = {}


def _ceil_to(x, m):
    return ((x + m - 1) // m) * m


def _part_major(w, kt):
    """[K, M] -> [128, kt*M] with each partition's kt-chunks contiguous."""
    k, m_dim = w.shape
    assert k == kt * 128
    return np.ascontiguousarray(
        w.reshape(kt, 128, m_dim).transpose(1, 0, 2).reshape(128, kt * m_dim))


def _build_program(TB):
    """Build the SPMD Tile program for tile-block size TB (multiple of 384)."""
    T = GPC * TB
    RC = TB // 128           # 128-row chunks per group
    KTI = IN // 128
    # L1 fp8-DR chunks (FD<=256) grouped into PSUM tiles of 512 output cols
    l1_full = TB // 512
    l1_tail = TB - l1_full * 512          # < 512, multiple of 128
    l1_ph0, l1_ph1 = [], []
    for k in range(l1_full):
        l1_ph0.append((k, 0, k * 512, 256))
        l1_ph1.append((k, 256, k * 512 + 256, 256))
    if l1_tail:
        t0 = min(l1_tail, 256)
        l1_ph0.append((l1_full, 0, l1_full * 512, t0))
        if l1_tail > t0:
            l1_ph1.append((l1_full, t0, l1_full * 512 + t0, l1_tail - t0))
    # L2 bf16 chunks of FD=384
    NC2 = TB // 384

    nc = bacc.Bacc("TRN2", target_bir_lowering=False, debug=False,
                   num_devices=N_CORES)

    xtq = nc.dram_tensor("xtq", [128, GPC * KTI * TB], FP8,
                         kind="ExternalInput")
    padb = nc.dram_tensor("padb", [128, GPC * RC], F32, kind="ExternalInput")
    xw = nc.dram_tensor("xw", [128, KTI * GPC], BF16, kind="ExternalInput")
    wl0q = nc.dram_tensor("wl0q", [128, KTI * GL], FP8, kind="ExternalInput")
    wl1e = nc.dram_tensor("wl1e", [128, (GL // 128) * LC], BF16,
                          kind="ExternalInput")
    wvq = nc.dram_tensor("wvq", [128, (LC // 128) * LC], FP8,
                         kind="ExternalInput")
    wklq = nc.dram_tensor("wklq", [128, (LC // 128) * L], FP8,
                          kind="ExternalInput")
    wg0 = nc.dram_tensor("wg0", [128, KTI * 2 * GL], BF16,
                         kind="ExternalInput")
    wg1 = nc.dram_tensor("wg1", [128, (2 * GL // 128) * GL], BF16,
                         kind="ExternalInput")
    bl0s = nc.dram_tensor("bl0s", [128, GL // 128], F32, kind="ExternalInput")
    bl1s = nc.dram_tensor("bl1s", [128, LC // 128], F32, kind="ExternalInput")
    bg0t = nc.dram_tensor("bg0t", [128, 2 * GL // 128], F32, kind="ExternalInput")
    bg1t = nc.dram_tensor("bg1t", [128, GL // 128], F32, kind="ExternalInput")
    out_og = nc.dram_tensor("out_og", [L, GPC, LC], F32, kind="ExternalOutput")
    out_w = nc.dram_tensor("out_w", [128, GL // 128, GPC], F32,
                           kind="ExternalOutput")

    tick = [0]

    def evac(out_ap, in_ap, bias_ap=None):
        """PSUM -> SBUF eviction; optional fused bias-add + relu.
        Alternates DVE / ACT to balance engine load."""
        use_dve = (tick[0] % 2 == 0)
        tick[0] += 1
        if bias_ap is None:
            if use_dve:
                nc.vector.tensor_copy(out_ap, in_ap)
            else:
                nc.scalar.copy(out_ap, in_ap)
        else:
            if use_dve:
                nc.vector.tensor_scalar(out_ap, in_ap, bias_ap, 0.0,
                                        op0=ALU.add, op1=ALU.max)
            else:
                nc.scalar.activation(out_ap, in_ap, ACTF.Relu, bias=bias_ap)

    with tile.TileContext(nc) as tc:
        with (
            tc.tile_pool(name="weights", bufs=1) as wpool,
            tc.tile_pool(name="wg", bufs=1) as wgpool,
        ):
            wl0_sb = wpool.tile([128, KTI, GL], FP8)
            nc.scalar.dma_start(out=wl0_sb,
                                in_=wl0q.ap().rearrange("p (kt m) -> p kt m",
                                                        kt=KTI))
            wl1_sb = wpool.tile([128, GL // 128, LC], BF16)
            nc.scalar.dma_start(out=wl1_sb,
                                in_=wl1e.ap().rearrange("p (kt m) -> p kt m",
                                                        kt=GL // 128))
            wv_sb = wpool.tile([128, LC // 128, LC], FP8)
            nc.scalar.dma_start(out=wv_sb,
                                in_=wvq.ap().rearrange("p (kt m) -> p kt m",
                                                       kt=LC // 128))
            wkl_sb = wpool.tile([128, LC // 128, L], FP8)
            nc.scalar.dma_start(out=wkl_sb,
                                in_=wklq.ap().rearrange("p (kt m) -> p kt m",
                                                        kt=LC // 128))
            bl0_sb = wpool.tile([128, GL // 128], F32)
            nc.scalar.dma_start(out=bl0_sb, in_=bl0s.ap())
            bl1_sb = wpool.tile([128, LC // 128], F32)
            nc.scalar.dma_start(out=bl1_sb, in_=bl1s.ap())
            padb_sb = wpool.tile([128, GPC * RC], F32)
            nc.scalar.dma_start(out=padb_sb, in_=padb.ap())
            # whole-branch inputs on the gpsimd queue (small ones first)
            xw_sb = wgpool.tile([128, KTI, GPC], BF16)
            nc.gpsimd.dma_start(out=xw_sb,
                                in_=xw.ap().rearrange("p (kt t) -> p kt t",
                                                      kt=KTI))
            bg0_sb = wgpool.tile([128, 2 * GL // 128], F32)
            nc.gpsimd.dma_start(out=bg0_sb, in_=bg0t.ap())
            bg1_sb = wgpool.tile([128, GL // 128], F32)
            nc.gpsimd.dma_start(out=bg1_sb, in_=bg1t.ap())
            wg0_sb = wgpool.tile([128, KTI, 2 * GL], BF16)
            nc.gpsimd.dma_start(out=wg0_sb,
                                in_=wg0.ap().rearrange("p (kt m) -> p kt m",
                                                       kt=KTI))
            wg1_sb = wgpool.tile([128, 2 * GL // 128, GL], BF16)
            nc.gpsimd.dma_start(out=wg1_sb,
                                in_=wg1.ap().rearrange("p (kt m) -> p kt m",
                                                       kt=2 * GL // 128))

            # ------- whole-instance branch first (WB == 1), fills DMA lag ----
            with (
                tc.tile_pool(name="wtile", bufs=1) as wtpool,
                tc.tile_pool(name="pw", bufs=2, space="PSUM") as pw,
            ):
                h1w_sb = wtpool.tile([128, 2 * GL // 128, GPC], BF16)
                for mc in range(2 * GL // 128):
                    ps = pw.tile([128, GPC], F32, tag="pw")
                    for kt in range(KTI):
                        nc.tensor.matmul(
                            ps, wg0_sb[:, kt, mc * 128:(mc + 1) * 128],
                            xw_sb[:, kt, :],
                            start=(kt == 0), stop=(kt == KTI - 1))
                    evac(h1w_sb[:, mc, :], ps, bg0_sb[:, mc:mc + 1])

                wag_sb = wtpool.tile([128, GL // 128, GPC], F32)
                for mc in range(GL // 128):
                    ps = pw.tile([128, GPC], F32, tag="pw")
                    for kt in range(2 * GL // 128):
                        nc.tensor.matmul(
                            ps, wg1_sb[:, kt, mc * 128:(mc + 1) * 128],
                            h1w_sb[:, kt, :],
                            start=(kt == 0), stop=(kt == 2 * GL // 128 - 1))
                    evac(wag_sb[:, mc, :], ps, bg1_sb[:, mc:mc + 1])
                nc.gpsimd.dma_start(out=out_w.ap(), in_=wag_sb)

            # ---------------- tile-instance branch, per group ----------------
            with (
                tc.tile_pool(name="xt", bufs=2) as xtpool,
                tc.tile_pool(name="h1", bufs=2) as h1pool,
                tc.tile_pool(name="xt2", bufs=2) as xt2pool,
                tc.tile_pool(name="vall", bufs=2) as vpool,
                tc.tile_pool(name="ext", bufs=2) as extpool,
                tc.tile_pool(name="small", bufs=2) as smpool,
                tc.tile_pool(name="ogall", bufs=1) as ogpool,
                tc.tile_pool(name="pA", bufs=3, space="PSUM") as pA,
                tc.tile_pool(name="pT", bufs=1, space="PSUM") as pT,
                tc.tile_pool(name="pV", bufs=2, space="PSUM") as pV,
                tc.tile_pool(name="pO", bufs=2, space="PSUM") as pO,
            ):
                og_sb = ogpool.tile([L, GPC, LC], F32)
                KP = IN // 256

                for j in range(GPC):
                    xt_sb = xtpool.tile([128, KTI, TB], FP8)
                    nc.sync.dma_start(
                        out=xt_sb,
                        in_=xtq.ap()[:, j * KTI * TB:(j + 1) * KTI * TB]
                            .rearrange("p (kt t) -> p kt t", kt=KTI))

                    # L1: h1' = relu(32768*(x@Wl0) + 32768*bl0)  [512, TB] bf16
                    h1_sb = h1pool.tile([128, GL // 128, TB], BF16)
                    for mc in range(GL // 128):
                        tiles = [pA.tile([128, 512], F32, tag="pa", name="pa")
                                 for _ in range(l1_full)]
                        if l1_tail:
                            tiles.append(pT.tile([128, 512], F32, tag="pt",
                                                 name="pt"))
                        for phase in (l1_ph0, l1_ph1):
                            for kp in range(KP):
                                st, sp = kp == 0, kp == KP - 1
                                for (tk, toff, ooff, w) in phase:
                                    nc.tensor.matmul(
                                        tiles[tk][:, toff:toff + w],
                                        wl0_sb[:, 2 * kp:2 * kp + 2,
                                               mc * 128:(mc + 1) * 128],
                                        xt_sb[:, 2 * kp:2 * kp + 2,
                                              ooff:ooff + w],
                                        start=st, stop=sp, perf_mode=DR)
                        for k in range(l1_full):
                            evac(h1_sb[:, mc, k * 512:(k + 1) * 512], tiles[k],
                                 bl0_sb[:, mc:mc + 1])
                        if l1_tail:
                            evac(h1_sb[:, mc, l1_full * 512:TB],
                                 tiles[-1][:, 0:l1_tail], bl0_sb[:, mc:mc + 1])

                    # L2: xt2' = relu(16*(h1@Wl1) + 16*bl1) [256, TB] fp8
                    xt2_sb = xt2pool.tile([128, LC // 128, TB], FP8)
                    for mc in range(LC // 128):
                        tiles = [pA.tile([128, 512], F32, tag="pa", name="pa")
                                 for _ in range(min(NC2, 2))]
                        for _ in range(NC2 - 2):
                            tiles.append(pT.tile([128, 512], F32, tag="pt",
                                                 name="pt"))
                        for kt in range(GL // 128):
                            st, sp = kt == 0, kt == GL // 128 - 1
                            for t in range(NC2):
                                nc.tensor.matmul(
                                    tiles[t][:, 0:384],
                                    wl1_sb[:, kt, mc * 128:(mc + 1) * 128],
                                    h1_sb[:, kt, t * 384:(t + 1) * 384],
                                    start=st, stop=sp)
                        for t in range(NC2):
                            evac(xt2_sb[:, mc, t * 384:(t + 1) * 384],
                                 tiles[t][:, 0:384], bl1_sb[:, mc:mc + 1])

                    # v' and transposed scores, per 128-row chunk:
                    # pv[:, 0:256]   = 16384 * v_rows
                    # pv[:, 256:264] = 8192 * scores_rows
                    vall_sb = vpool.tile([128, RC, 1 + LC + L], BF16)
                    nc.vector.memset(vall_sb[:, :, 0:1], 1.0)
                    ext_sb = extpool.tile([128, RC, L], BF16)
                    for rc in range(RC):
                        r0 = rc * 128
                        pv = pV.tile([128, LC + L], F32, tag="pv")
                        nc.tensor.matmul(pv[:, 0:LC],
                                         xt2_sb[:, 0:2, r0:r0 + 128],
                                         wv_sb[:, 0:2, :],
                                         start=True, stop=True, perf_mode=DR)
                        nc.tensor.matmul(pv[:, LC:LC + L],
                                         xt2_sb[:, 0:2, r0:r0 + 128],
                                         wkl_sb[:, 0:2, :],
                                         start=True, stop=True, perf_mode=DR)
                        evac(vall_sb[:, rc, 1:1 + LC + L], pv)
                        # ex = exp(scores + padkill) (bf16); pads -> 0
                        nc.scalar.activation(
                            ext_sb[:, rc, :], vall_sb[:, rc, 1 + LC:1 + LC + L],
                            ACTF.Exp, scale=1.0 / (SH2 * SKL),
                            bias=padb_sb[:, j * RC + rc:j * RC + rc + 1])

                    # out_cat[l, 0] = denom, out_cat[l, 1:] = 16384*sum ex*v
                    po = pO.tile([L, 1 + LC], F32, tag="po")
                    for rc in range(RC):
                        nc.tensor.matmul(po, ext_sb[:, rc, :],
                                         vall_sb[:, rc, 0:1 + LC],
                                         start=(rc == 0), stop=(rc == RC - 1))
                    rden = smpool.tile([L, 1], F32, tag="rden")
                    nc.vector.reciprocal(rden, po[:, 0:1])
                    nc.vector.tensor_scalar(og_sb[:, j, :], po[:, 1:1 + LC],
                                            rden, 1.0 / (SH2 * SWV),
                                            op0=ALU.mult, op1=ALU.mult)
                    nc.sync.dma_start(out=out_og.ap()[:, j, :],
                                      in_=og_sb[:, j, :])

    nc.compile()
    return nc


def _get_program(key):
    if key not in _prog_cache:
        _prog_cache[key] = _build_program(key)
    return _prog_cache[key]


def kernel(**inputs):
    x = np.ascontiguousarray(np.asarray(inputs["x"], dtype=np.float32))
    group = np.asarray(inputs["group"]).astype(np.int64)
    itype = np.asarray(inputs["instance_type"]).astype(np.int64)
    Wl0 = np.asarray(inputs["Wl0"], np.float32)
    bl0 = np.asarray(inputs["bl0"], np.float32)
    Wl1 = np.asarray(inputs["Wl1"], np.float32)
    bl1 = np.asarray(inputs["bl1"], np.float32)
    Wg0 = np.asarray(inputs["Wg0"], np.float32)
    bg0 = np.asarray(inputs["bg0"], np.float32)
    Wg1 = np.asarray(inputs["Wg1"], np.float32)
    bg1 = np.asarray(inputs["bg1"], np.float32)
    Wk = np.asarray(inputs["Wk"], np.float32)
    Wv = np.asarray(inputs["Wv"], np.float32)
    bv = np.asarray(inputs["bv"], np.float32)
    latent = np.asarray(inputs["latent"], np.float32)
    Wout = np.asarray(inputs["Wout"], np.float32)
    bout = np.asarray(inputs["bout"], np.float32)

    # ---- host bucketing ----
    is_tile = itype == 1
    is_whole = itype == 0
    tile_idx = [np.where(is_tile & (group == g))[0] for g in range(G)]
    whole_idx = [np.where(is_whole & (group == g))[0] for g in range(G)]
    ng = np.array([len(ix) for ix in tile_idx])
    wg = np.array([len(ix) for ix in whole_idx])
    assert (wg == 1).all(), "kernel assumes exactly one whole instance per group"
    TB = max(384, _ceil_to(int(ng.max()), 384))
    T = GPC * TB
    RC = TB // 128
    KTI = IN // 128

    # ---- per-core staged arrays (partition-major, contiguous runs) ----
    scale = 1.0 / math.sqrt(LC)
    wkl = (Wk @ latent.T) * scale                        # [LC, L]
    shared = dict(
        wl0q=_part_major(Wl0 * SW0, KTI).astype(NP_FP8),
        wl1e=_part_major(Wl1 * (SH2 / C1), GL // 128).astype(NP_BF16),
        wvq=_part_major(Wv * SWV, LC // 128).astype(NP_FP8),
        wklq=_part_major(wkl * SKL, LC // 128).astype(NP_FP8),
        wg0=_part_major(Wg0, KTI).astype(NP_BF16),
        wg1=_part_major(Wg1, 2 * GL // 128).astype(NP_BF16),
        bl0s=np.ascontiguousarray((bl0 * C1).reshape(-1, 128).T),
        bl1s=np.ascontiguousarray((bl1 * SH2).reshape(-1, 128).T),
        bg0t=np.ascontiguousarray(bg0.reshape(-1, 128).T),
        bg1t=np.ascontiguousarray(bg1.reshape(-1, 128).T),
    )
    in_maps = []
    for c in range(N_CORES):
        xtf = np.zeros((IN, T), np.float32)
        xwf = np.zeros((IN, GPC), np.float32)
        pb = np.full((128, GPC * RC), PADB, np.float32)
        for j in range(GPC):
            g = c * GPC + j
            ti, wi = tile_idx[g], whole_idx[g]
            xtf[:, j * TB:j * TB + len(ti)] = x[ti].T * SX
            xwf[:, j:j + 1] = x[wi].T
            n = len(ti)
            full_rc, rem = divmod(n, 128)
            pb[:, j * RC:j * RC + full_rc] = 0.0
            if rem:
                pb[:rem, j * RC + full_rc] = 0.0
        # [IN, T] -> [128, GPC*KTI*TB]: partition p, group j, kt, t contiguous
        xtq = np.ascontiguousarray(
            xtf.reshape(KTI, 128, GPC, TB).transpose(1, 2, 0, 3)
               .reshape(128, GPC * KTI * TB))
        xwq = np.ascontiguousarray(
            xwf.reshape(KTI, 128, GPC).transpose(1, 0, 2).reshape(128, -1))
        in_maps.append(dict(xtq=xtq.astype(NP_FP8), padb=pb,
                            xw=xwq.astype(NP_BF16), **shared))

    nc = _get_program(TB)
    trace = os.environ.get("KERNEL_TRACE") == "1"
    if trace:
        _install_ntff_shim()
    res = run_bass_kernel_spmd(nc, in_maps, core_ids=list(range(N_CORES)),
                               trace=trace)
    global last_exec_time_ns, last_mean_exec_time_ns
    last_exec_time_ns = res.exec_time_ns
    last_mean_exec_time_ns = res.mean_exec_time_ns

    # ---- host assembly ----
    whole_agg = np.empty((G, GL), np.float32)
    out_group = np.empty((G, L, LC), np.float32)
    for c in range(N_CORES):
        ow = np.asarray(res.results[c]["out_w"], np.float32)
        og = np.asarray(res.results[c]["out_og"], np.float32)
        wa = ow.transpose(1, 0, 2).reshape(GL, GPC)
        for j in range(GPC):
            g = c * GPC + j
            whole_agg[g] = wa[:, j]
            out_group[g] = og[:, j, :] + bv[None, :]
    fused = np.concatenate([whole_agg, out_group.reshape(G, L * LC)], axis=1)
    return (fused @ Wout + bout).astype(np.float32)
